# revision 18
# baseline (speedup 1.0000x reference)
"""Trainium2 Bass kernel for nn_FeatureContraction.

Computes out[b,c,w,x,v] = sum_i x[b,c,w,x,v,i] * node_attributes[b,c,i]
with B=C=128, X=3, Y=16 (wxv = 3*16*16 = 768, i = 16).

Strategy (8 NeuronCores, data-parallel over b, bandwidth-asymmetric):
  - the 8 NCs on this chip have measurably different sustained HBM
    read bandwidth under full load: odd NCs ~425 GB/s, nc0 ~330 GB/s,
    the other even NCs 335-430 (run-dependent).  SPMD model index
    preserves NC parity, so the shard is asymmetric: even models
    process 14 b-slices, odd models 18 (14 unconditional + 4 inside a
    `tc.If(is_odd == 1)` block at the end of the program; the branch
    condition is snapped into registers during the startup DMA window
    so the branch itself costs ~0.3us per engine).
  - SBUF layout: partitions = c (128), free = contiguous (wxv, i).
    Bulk x loads go through the SWDGE queue with an inline f32->bf16
    cast as full 6 MiB per-slice transfers (big packets sustain the
    highest SWDGE rate).  The SWDGE Q7 pipeline takes ~9 us to emit
    its first descriptors, so the first two eighth-chunks of slice 0
    are loaded as raw f32 via the two HWDGE rings (sync + scalar,
    first byte ~0.6 us) and multiplied in f32.
  - multiply: tmp[c, w, i] = x[c, w, i] * na[c, i] with a step-0
    broadcast AP on na.
  - reduce over i, split by w to balance engines:
      w < RED_SPLIT: DVE grouped tensor_reduce (innermost axis)
      w >= RED_SPLIT: 16 identity-weight PE matmuls accumulating the
      strided i-slices into PSUM, then ACT copies PSUM->SBUF.
  - output stored as bf16 (tolerance is 2e-2; halves the HBM write
    traffic), cast back to f32 on the host after the gather.  Writes
    near the stream end go through the (by then idle) SWDGE queue so
    the slow HWDGE out-ring cannot stretch the tail.
  - the last two slices of each stream (slices 12/13 for even models,
    extra slice 3 for odd models) are loaded as quarter chunks with
    interleaved PE/DVE assignment, so the post-DMA pipeline tail is
    one small mult+reduce instead of a full slice of matmul backlog.
"""

import sys

for _p in ("/opt/trn_rl_repo",):
    if _p not in sys.path:
        sys.path.append(_p)

import numpy as np

import concourse.bass as bass
import concourse.mybir as mybir
import concourse.tile as tile
from concourse import bacc
from concourse.bass_utils import run_bass_kernel_spmd

# Problem dims (hardcoded per spec)
B, C, X, Y = 128, 128, 3, 16
WXV = X * Y * Y          # 768
I = Y                    # 16 (contraction axis)
N_CORES = 8
B_MAIN = 14              # unconditional b-slices per core
B_EXTRA = 4              # extra b-slices on odd (fast) models
B_TOT = B_MAIN + B_EXTRA
# per-core slice counts by model parity: 4*14 + 4*18 = 128 = B
SIZES = [B_MAIN + B_EXTRA * (k % 2) for k in range(N_CORES)]
OFFS = np.cumsum([0] + SIZES).tolist()
assert OFFS[-1] == B

RED_SPLIT = 320          # DVE reduces w < RED_SPLIT, PE reduces the rest
E8 = 192                 # quarter-chunk width for the HWDGE warm-up loads
QW = WXV // 4            # 192-wide quarter chunks for the tail slices

F32 = mybir.dt.float32
BF16 = mybir.dt.bfloat16

_COMPILED = None


def _build():
    nc = bacc.Bacc("TRN2", target_bir_lowering=False, debug=False,
                   num_devices=N_CORES)

    x_d = nc.dram_tensor("x", [B_MAIN, C, WXV, I], F32, kind="ExternalInput")
    xe_d = nc.dram_tensor("xe", [B_EXTRA, C, WXV, I], F32,
                          kind="ExternalInput")
    na_d = nc.dram_tensor("naT", [C, B_TOT, I], F32, kind="ExternalInput")
    eye_d = nc.dram_tensor("eye", [C, C], F32, kind="ExternalInput")
    out_d = nc.dram_tensor("out", [B_MAIN, C, WXV], BF16,
                           kind="ExternalOutput")
    oute_d = nc.dram_tensor("oute", [B_EXTRA, C, WXV], BF16,
                            kind="ExternalOutput")

    WA = RED_SPLIT
    WB = WXV - RED_SPLIT

    with tile.TileContext(nc) as tc:
        with (
            tc.tile_pool(name="const", bufs=1) as constp,
            tc.tile_pool(name="xfp", bufs=3) as xfp,
            tc.tile_pool(name="x8p", bufs=2) as x8p,
            tc.tile_pool(name="xqp", bufs=3) as xqp,
            tc.tile_pool(name="tmpap", bufs=3) as tmpap,
            tc.tile_pool(name="tmpbp", bufs=3) as tmpbp,
            tc.tile_pool(name="outp", bufs=3) as outp,
            tc.tile_pool(name="psp", bufs=4, space="PSUM") as psp,
        ):
            eye = constp.tile([C, C], BF16)
            na_sb = constp.tile([C, B_TOT, I], BF16)
            eye_f = constp.tile([C, C], F32)
            na_f = constp.tile([C, B_TOT, I], F32)

            # ---- warm-up: consts + first two eighths of slice 0 via the
            # two HWDGE rings (first byte ~0.6us; Q7/SWDGE needs ~9us) ----
            nc.sync.dma_start(na_f[:], na_d[:])
            x80 = x8p.tile([C, E8, I], F32, tag="x8")
            nc.sync.dma_start(x80[:], x_d[0, :, 0:E8, :])
            x81 = x8p.tile([C, E8, I], F32, tag="x8")
            nc.scalar.dma_start(x81[:], x_d[0, :, E8:2 * E8, :])
            nc.scalar.dma_start(eye_f[:], eye_d[:])
            # slice-0 remainder starts the SWDGE stream immediately
            x0r = xfp.tile([C, WXV - 2 * E8, I], BF16, tag="xf")
            nc.gpsimd.dma_start(x0r[:], x_d[0, :, 2 * E8:, :])

            # snap the branch condition into registers now (engines are
            # idle waiting on the first loads); the If at the end then
            # costs one compare+branch instead of multi-us reg loads
            nc.cache_partition_id()
            pid = nc.partition_id()
            is_odd = nc.snap(pid % 2, min_val=0, max_val=1)

            nc.vector.tensor_copy(na_sb[:], na_f[:])
            nc.vector.tensor_copy(eye[:], eye_f[:])

            def mul_red(xt_ap, na_row, o_ap, w, na_fp32=False):
                """DVE: tmp = x*na (bf16 out), then grouped reduce over i."""
                srcna = na_f if na_fp32 else na_sb
                nab = srcna[:, na_row, :][:, None, :]
                t = tmpap.tile([C, w, I], BF16, tag="tmpa")
                nc.vector.tensor_mul(t[:], xt_ap, nab.broadcast_to([C, w, I]))
                with nc.allow_low_precision(reason="bf16 out, tol 2e-2"):
                    nc.vector.tensor_reduce(o_ap, t[:], mybir.AxisListType.X,
                                            mybir.AluOpType.add)

            def mul_pe(xt_ap, na_row, o_ap, w):
                """DVE mult then PE identity-matmul reduce, ACT copy out."""
                nab = na_sb[:, na_row, :][:, None, :]
                t = tmpbp.tile([C, w, I], BF16, tag="tmpb")
                nc.vector.tensor_mul(t[:], xt_ap, nab.broadcast_to([C, w, I]))
                ps = psp.tile([C, w], F32, tag="ps")
                for i in range(I):
                    nc.tensor.matmul(ps[:], eye[:], t[:, :, i],
                                     start=(i == 0), stop=(i == I - 1))
                nc.scalar.copy(o_ap, ps[:])

            # ---- slice 0: two f32 quarters (HWDGE) + all-PE remainder ----
            ot0 = outp.tile([C, WXV], BF16, tag="out")
            mul_red(x80[:], 0, ot0[:, 0:E8], E8, na_fp32=True)
            mul_red(x81[:], 0, ot0[:, E8:2 * E8], E8, na_fp32=True)
            mul_pe(x0r[:], 0, ot0[:, 2 * E8:], WXV - 2 * E8)
            nc.scalar.dma_start(out_d[0], ot0[:])

            def full_slice(src, na_row, odst, wq=None):
                """Load + process one full slice (one 6 MiB SWDGE load)."""
                xt = xfp.tile([C, WXV, I], BF16, tag="xf")
                nc.gpsimd.dma_start(xt[:], src)
                ot = outp.tile([C, WXV], BF16, tag="out")
                mul_pe(xt[:, RED_SPLIT:, :], na_row, ot[:, RED_SPLIT:], WB)
                mul_red(xt[:, :RED_SPLIT, :], na_row, ot[:, :RED_SPLIT], WA)
                (wq or nc.scalar).dma_start(odst, ot[:])

            def quartered_slice(srcb, na_row, odstb):
                """Load + process one slice as 4 quarter chunks with
                interleaved PE/DVE roles; out written in halves via the
                idle SWDGE queue.  Keeps the post-DMA tail to one small
                mult+reduce."""
                ot = outp.tile([C, WXV], BF16, tag="out")
                for q, use_pe in ((0, True), (1, False), (2, False),
                                  (3, False)):
                    xt = xqp.tile([C, QW, I], BF16, tag="xq")
                    nc.gpsimd.dma_start(xt[:], srcb[:, q * QW:(q + 1) * QW, :])
                    oq = ot[:, q * QW:(q + 1) * QW]
                    if use_pe:
                        mul_pe(xt[:], na_row, oq, QW)
                    else:
                        mul_red(xt[:], na_row, oq, QW)
                    if q == 1:
                        nc.gpsimd.dma_start(odstb[:, :2 * QW], ot[:, :2 * QW])
                nc.gpsimd.dma_start(odstb[:, 2 * QW:], ot[:, 2 * QW:])

            # ---- middle slices: full 6 MiB loads; the last ones write
            # their outputs through the (by then idle) SWDGE queue ----
            for b in range(1, B_MAIN - 2):
                wq = nc.gpsimd if b >= B_MAIN - 4 else None
                full_slice(x_d[b], b, out_d[b], wq)

            # ---- last unconditional slices: one more full, then the
            # final slice quartered (PE only gets the first quarter, so
            # its matmul backlog clears while DVE runs the short tail) ----
            full_slice(x_d[B_MAIN - 2], B_MAIN - 2, out_d[B_MAIN - 2],
                       nc.gpsimd)
            quartered_slice(x_d[B_MAIN - 1], B_MAIN - 1, out_d[B_MAIN - 1])

            # ---- conditional extras at the very end: even cores skip
            # with one cheap branch; odd cores keep streaming, ending in
            # a quartered tail slice ----
            with tc.If(is_odd == 1):
                for e in range(B_EXTRA - 1):
                    wq = nc.gpsimd if e == B_EXTRA - 2 else None
                    full_slice(xe_d[e], B_MAIN + e, oute_d[e], wq)
                E3 = B_EXTRA - 1
                quartered_slice(xe_d[E3], B_MAIN + E3, oute_d[E3])

    nc.compile()
    return nc


def _get_compiled():
    global _COMPILED
    if _COMPILED is None:
        _COMPILED = _build()
    return _COMPILED


def _make_in_maps(inputs: dict):
    x = np.ascontiguousarray(np.asarray(inputs["x"], dtype=np.float32))
    na = np.asarray(inputs["node_attributes"], dtype=np.float32)

    x_sh = x.reshape(B, C, WXV, I)
    naT = np.ascontiguousarray(na.transpose(1, 0, 2))  # [C, B, I]
    eye = np.eye(C, dtype=np.float32)
    xe_zero = np.zeros((B_EXTRA, C, WXV, I), np.float32)

    in_maps = []
    for k in range(N_CORES):
        b0, n = OFFS[k], SIZES[k]
        na_k = np.zeros((C, B_TOT, I), np.float32)
        na_k[:, :n, :] = naT[:, b0:b0 + n, :]
        in_maps.append(
            {
                "x": x_sh[b0:b0 + B_MAIN],
                "xe": (np.ascontiguousarray(x_sh[b0 + B_MAIN:b0 + n])
                       if n > B_MAIN else xe_zero),
                "naT": na_k,
                "eye": eye,
            }
        )
    return in_maps


def _gather(results) -> np.ndarray:
    parts = []
    for k, r in enumerate(results):
        parts.append(np.asarray(r["out"]))
        if SIZES[k] > B_MAIN:
            parts.append(np.asarray(r["oute"]))
    out = np.concatenate(parts, axis=0)
    return out.astype(np.float32).reshape(B, C, X, Y, Y)


def _run(inputs: dict, trace: bool = False, trace_cores=None):
    in_maps = _make_in_maps(inputs)
    nc = _get_compiled()
    res = run_bass_kernel_spmd(
        nc,
        in_maps,
        core_ids=list(range(N_CORES)),
        trace=trace,
        trace_cores=trace_cores,
    )
    return _gather(res.results), res


def kernel(**inputs) -> np.ndarray:
    out, _ = _run(inputs, trace=False)
    return out


# revision 19
# speedup vs baseline: 1.0038x; 1.0038x over previous
"""Trainium2 Bass kernel for nn_FeatureContraction.

Computes out[b,c,w,x,v] = sum_i x[b,c,w,x,v,i] * node_attributes[b,c,i]
with B=C=128, X=3, Y=16 (wxv = 3*16*16 = 768, i = 16).

Strategy (8 NeuronCores, data-parallel over b, bandwidth-asymmetric):
  - the 8 NCs on this chip have measurably different sustained HBM
    read bandwidth under full load: odd NCs ~425 GB/s, nc0 ~330 GB/s,
    the other even NCs 335-430 (run-dependent).  SPMD model index
    preserves NC parity, so the shard is asymmetric: even models
    process 14 b-slices, odd models 18 (14 unconditional + 4 inside a
    `tc.If(is_odd == 1)` block at the end of the program; the branch
    condition is snapped into registers during the startup DMA window
    so the branch itself costs ~0.3us per engine).
  - SBUF layout: partitions = c (128), free = contiguous (wxv, i).
    Bulk x loads go through the SWDGE queue with an inline f32->bf16
    cast as full 6 MiB per-slice transfers (big packets sustain the
    highest SWDGE rate).  The SWDGE Q7 pipeline takes ~9 us to emit
    its first descriptors, so the first two eighth-chunks of slice 0
    are loaded as raw f32 via the two HWDGE rings (sync + scalar,
    first byte ~0.6 us) and multiplied in f32.
  - multiply: tmp[c, w, i] = x[c, w, i] * na[c, i] with a step-0
    broadcast AP on na.
  - reduce over i, split by w to balance engines:
      w < RED_SPLIT: DVE grouped tensor_reduce (innermost axis)
      w >= RED_SPLIT: 16 identity-weight PE matmuls accumulating the
      strided i-slices into PSUM, then ACT copies PSUM->SBUF.
  - output stored as bf16 (tolerance is 2e-2; halves the HBM write
    traffic), cast back to f32 on the host after the gather.  Writes
    near the stream end go through the (by then idle) SWDGE queue so
    the slow HWDGE out-ring cannot stretch the tail.
  - the last two slices of each stream (slices 12/13 for even models,
    extra slice 3 for odd models) are loaded as quarter chunks with
    interleaved PE/DVE assignment, so the post-DMA pipeline tail is
    one small mult+reduce instead of a full slice of matmul backlog.
"""

import sys

for _p in ("/opt/trn_rl_repo",):
    if _p not in sys.path:
        sys.path.append(_p)

import numpy as np

import concourse.bass as bass
import concourse.mybir as mybir
import concourse.tile as tile
from concourse import bacc
from concourse.bass_utils import run_bass_kernel_spmd

# Problem dims (hardcoded per spec)
B, C, X, Y = 128, 128, 3, 16
WXV = X * Y * Y          # 768
I = Y                    # 16 (contraction axis)
N_CORES = 8
B_MAIN = 14              # unconditional b-slices per core
B_EXTRA = 4              # extra b-slices on odd (fast) models
B_TOT = B_MAIN + B_EXTRA
# per-core slice counts by model parity: 4*14 + 4*18 = 128 = B
SIZES = [B_MAIN + B_EXTRA * (k % 2) for k in range(N_CORES)]
OFFS = np.cumsum([0] + SIZES).tolist()
assert OFFS[-1] == B

RED_SPLIT = 320          # DVE reduces w < RED_SPLIT, PE reduces the rest
E8 = 192                 # quarter-chunk width for the HWDGE warm-up loads
QW = WXV // 4            # 192-wide quarter chunks for the tail slices

F32 = mybir.dt.float32
BF16 = mybir.dt.bfloat16

_COMPILED = None


def _build():
    nc = bacc.Bacc("TRN2", target_bir_lowering=False, debug=False,
                   num_devices=N_CORES)

    x_d = nc.dram_tensor("x", [B_MAIN, C, WXV, I], F32, kind="ExternalInput")
    xe_d = nc.dram_tensor("xe", [B_EXTRA, C, WXV, I], F32,
                          kind="ExternalInput")
    na_d = nc.dram_tensor("naT", [C, B_TOT, I], F32, kind="ExternalInput")
    eye_d = nc.dram_tensor("eye", [C, C], F32, kind="ExternalInput")
    out_d = nc.dram_tensor("out", [B_MAIN, C, WXV], BF16,
                           kind="ExternalOutput")
    oute_d = nc.dram_tensor("oute", [B_EXTRA, C, WXV], BF16,
                            kind="ExternalOutput")

    WA = RED_SPLIT
    WB = WXV - RED_SPLIT

    with tile.TileContext(nc) as tc:
        with (
            tc.tile_pool(name="const", bufs=1) as constp,
            tc.tile_pool(name="xfp", bufs=3) as xfp,
            tc.tile_pool(name="x8p", bufs=2) as x8p,
            tc.tile_pool(name="xqp", bufs=3) as xqp,
            tc.tile_pool(name="tmpap", bufs=3) as tmpap,
            tc.tile_pool(name="tmpbp", bufs=3) as tmpbp,
            tc.tile_pool(name="outp", bufs=4) as outp,
            tc.tile_pool(name="psp", bufs=6, space="PSUM") as psp,
        ):
            eye = constp.tile([C, C], BF16)
            na_sb = constp.tile([C, B_TOT, I], BF16)
            eye_f = constp.tile([C, C], F32)
            na_f = constp.tile([C, B_TOT, I], F32)

            # ---- warm-up: consts + first two eighths of slice 0 via the
            # two HWDGE rings (first byte ~0.6us; Q7/SWDGE needs ~9us) ----
            nc.sync.dma_start(na_f[:], na_d[:])
            x80 = x8p.tile([C, E8, I], F32, tag="x8")
            nc.sync.dma_start(x80[:], x_d[0, :, 0:E8, :])
            x81 = x8p.tile([C, E8, I], F32, tag="x8")
            nc.scalar.dma_start(x81[:], x_d[0, :, E8:2 * E8, :])
            nc.scalar.dma_start(eye_f[:], eye_d[:])
            # slice-0 remainder starts the SWDGE stream immediately
            x0r = xfp.tile([C, WXV - 2 * E8, I], BF16, tag="xf")
            nc.gpsimd.dma_start(x0r[:], x_d[0, :, 2 * E8:, :])

            # snap the branch condition into registers now (engines are
            # idle waiting on the first loads); the If at the end then
            # costs one compare+branch instead of multi-us reg loads
            nc.cache_partition_id()
            pid = nc.partition_id()
            is_odd = nc.snap(pid % 2, min_val=0, max_val=1)

            nc.vector.tensor_copy(na_sb[:], na_f[:])
            nc.vector.tensor_copy(eye[:], eye_f[:])

            def mul_red(xt_ap, na_row, o_ap, w, na_fp32=False):
                """DVE: tmp = x*na (bf16 out), then grouped reduce over i."""
                srcna = na_f if na_fp32 else na_sb
                nab = srcna[:, na_row, :][:, None, :]
                t = tmpap.tile([C, w, I], BF16, tag="tmpa")
                nc.vector.tensor_mul(t[:], xt_ap, nab.broadcast_to([C, w, I]))
                with nc.allow_low_precision(reason="bf16 out, tol 2e-2"):
                    nc.vector.tensor_reduce(o_ap, t[:], mybir.AxisListType.X,
                                            mybir.AluOpType.add)

            def mul_pe(xt_ap, na_row, o_ap, w):
                """DVE mult then PE identity-matmul reduce, ACT copy out."""
                nab = na_sb[:, na_row, :][:, None, :]
                t = tmpbp.tile([C, w, I], BF16, tag="tmpb")
                nc.vector.tensor_mul(t[:], xt_ap, nab.broadcast_to([C, w, I]))
                ps = psp.tile([C, w], F32, tag="ps")
                for i in range(I):
                    nc.tensor.matmul(ps[:], eye[:], t[:, :, i],
                                     start=(i == 0), stop=(i == I - 1))
                nc.scalar.copy(o_ap, ps[:])

            # ---- slice 0: two f32 quarters (HWDGE) + all-PE remainder ----
            ot0 = outp.tile([C, WXV], BF16, tag="out")
            mul_red(x80[:], 0, ot0[:, 0:E8], E8, na_fp32=True)
            mul_red(x81[:], 0, ot0[:, E8:2 * E8], E8, na_fp32=True)
            mul_pe(x0r[:], 0, ot0[:, 2 * E8:], WXV - 2 * E8)
            nc.scalar.dma_start(out_d[0], ot0[:])

            def full_slice(src, na_row, odst, wq=None):
                """Load + process one full slice (one 6 MiB SWDGE load)."""
                xt = xfp.tile([C, WXV, I], BF16, tag="xf")
                nc.gpsimd.dma_start(xt[:], src)
                ot = outp.tile([C, WXV], BF16, tag="out")
                mul_pe(xt[:, RED_SPLIT:, :], na_row, ot[:, RED_SPLIT:], WB)
                mul_red(xt[:, :RED_SPLIT, :], na_row, ot[:, :RED_SPLIT], WA)
                (wq or nc.scalar).dma_start(odst, ot[:])

            def quartered_slice(srcb, na_row, odstb):
                """Load + process one slice as 4 quarter chunks with
                interleaved PE/DVE roles; out written in halves via the
                idle SWDGE queue.  Keeps the post-DMA tail to one small
                mult+reduce."""
                ot = outp.tile([C, WXV], BF16, tag="out")
                for q, use_pe in ((0, True), (1, False), (2, False),
                                  (3, False)):
                    xt = xqp.tile([C, QW, I], BF16, tag="xq")
                    nc.gpsimd.dma_start(xt[:], srcb[:, q * QW:(q + 1) * QW, :])
                    oq = ot[:, q * QW:(q + 1) * QW]
                    if use_pe:
                        mul_pe(xt[:], na_row, oq, QW)
                    else:
                        mul_red(xt[:], na_row, oq, QW)
                    if q == 1:
                        nc.gpsimd.dma_start(odstb[:, :2 * QW], ot[:, :2 * QW])
                nc.gpsimd.dma_start(odstb[:, 2 * QW:], ot[:, 2 * QW:])

            # ---- middle slices: full 6 MiB loads; the last ones write
            # their outputs through the (by then idle) SWDGE queue ----
            for b in range(1, B_MAIN - 2):
                wq = nc.gpsimd if b >= B_MAIN - 4 else None
                full_slice(x_d[b], b, out_d[b], wq)

            # ---- last unconditional slices: one more full, then the
            # final slice quartered (PE only gets the first quarter, so
            # its matmul backlog clears while DVE runs the short tail) ----
            full_slice(x_d[B_MAIN - 2], B_MAIN - 2, out_d[B_MAIN - 2],
                       nc.gpsimd)
            quartered_slice(x_d[B_MAIN - 1], B_MAIN - 1, out_d[B_MAIN - 1])

            # ---- conditional extras at the very end: even cores skip
            # with one cheap branch; odd cores keep streaming, ending in
            # a quartered tail slice ----
            with tc.If(is_odd == 1):
                for e in range(B_EXTRA - 1):
                    wq = nc.gpsimd if e == B_EXTRA - 2 else None
                    full_slice(xe_d[e], B_MAIN + e, oute_d[e], wq)
                E3 = B_EXTRA - 1
                quartered_slice(xe_d[E3], B_MAIN + E3, oute_d[E3])

    nc.compile()
    return nc


def _get_compiled():
    global _COMPILED
    if _COMPILED is None:
        _COMPILED = _build()
    return _COMPILED


def _make_in_maps(inputs: dict):
    x = np.ascontiguousarray(np.asarray(inputs["x"], dtype=np.float32))
    na = np.asarray(inputs["node_attributes"], dtype=np.float32)

    x_sh = x.reshape(B, C, WXV, I)
    naT = np.ascontiguousarray(na.transpose(1, 0, 2))  # [C, B, I]
    eye = np.eye(C, dtype=np.float32)
    xe_zero = np.zeros((B_EXTRA, C, WXV, I), np.float32)

    in_maps = []
    for k in range(N_CORES):
        b0, n = OFFS[k], SIZES[k]
        na_k = np.zeros((C, B_TOT, I), np.float32)
        na_k[:, :n, :] = naT[:, b0:b0 + n, :]
        in_maps.append(
            {
                "x": x_sh[b0:b0 + B_MAIN],
                "xe": (np.ascontiguousarray(x_sh[b0 + B_MAIN:b0 + n])
                       if n > B_MAIN else xe_zero),
                "naT": na_k,
                "eye": eye,
            }
        )
    return in_maps


def _gather(results) -> np.ndarray:
    parts = []
    for k, r in enumerate(results):
        parts.append(np.asarray(r["out"]))
        if SIZES[k] > B_MAIN:
            parts.append(np.asarray(r["oute"]))
    out = np.concatenate(parts, axis=0)
    return out.astype(np.float32).reshape(B, C, X, Y, Y)


def _run(inputs: dict, trace: bool = False, trace_cores=None):
    in_maps = _make_in_maps(inputs)
    nc = _get_compiled()
    res = run_bass_kernel_spmd(
        nc,
        in_maps,
        core_ids=list(range(N_CORES)),
        trace=trace,
        trace_cores=trace_cores,
    )
    return _gather(res.results), res


def kernel(**inputs) -> np.ndarray:
    out, _ = _run(inputs, trace=False)
    return out
